# revision 1
# baseline (speedup 1.0000x reference)
"""BFP-quantized 3x3 conv (stride 1, pad 1) as on-the-fly im2col matmul on
8 TRN2 cores, using fp8 DoubleRow matmuls (2 k-tiles per instruction at 0.5
cycles/row = 4x bf16 PE throughput).

Shapes (hardcoded): inputs [32,128,56,56] f32, weight [256,128,3,3] f32,
bias [256] f32 -> out [32,256,56,56] f32.

Strategy: data-parallel over batch (4 images per core). The reference
quantizes both operands to 8-bit-mantissa BFP; we approximate it as:

  out = a8 @ (w_hi + w_lo) + ea8 @ w_hi[compensated positions]

where
  - w_hi + w_lo == qw EXACTLY: the BFP weights' 8-bit mantissas are split
    into two 4-bit nibbles, each exactly representable in fp8 e4m3 after a
    global 2^9 scaling (power-of-two, undone on the host).
  - a8 = e4m3(x) quantized once per input pixel (so im2col can be done
    on the fly from shifted SBUF views -> no 9x HBM blowup),
  - ea8 = e4m3(x - a8) is an fp8 error-compensation plane that cancels the
    activation rounding error on NPOS of the 9 kernel positions.

Each DoubleRow matmul contracts 2 of the (18 + NPOS) k-tiles. PSUM
accumulates in f32; outputs stored f16 (scaled by 2^9), descaled + bias
added on host.
"""

import numpy as np
import ml_dtypes

import concourse.bacc as bacc
import concourse.mybir as mybir
from concourse.tile import TileContext
from concourse.bass_utils import run_bass_kernel_spmd
from bass_rust import AP

FP8 = ml_dtypes.float8_e4m3

N_CORES = 8
N_IMG, C_IN, H, W = 32, 128, 56, 56
C_OUT, KS = 256, 3
IMG_PER_CORE = N_IMG // N_CORES   # 4
PIX = H * W                       # 3136
M = IMG_PER_CORE * PIX            # 12544 output columns per core
K = C_IN * KS * KS                # 1152

HP = H + 2                        # 58 padded
PLANE = HP * HP                   # 3364 elements per partition per plane
ZPAD = 512                        # zero tail per image block (dummy k-tile)
IMG_STRIDE = 2 * PLANE + ZPAD     # 7240
ZOFF = 2 * PLANE                  # zero region offset within image block

ROWS = 8                          # output rows per matmul chunk
MCHUNK = ROWS * W                 # 448 moving rows per DR matmul
NOHB = H // ROWS                  # 7 chunks per image

WSCALE = 512.0                    # global 2^9 weight scaling for fp8 exactness

M_BIT, BLOCK = 8, 64

# k-tile kinds
HI_A, LO_A, HI_E, ZERO = 0, 1, 2, 3

# 27 k-tiles + 1 zero tile in 14 DoubleRow pairs (full activation
# compensation -> measured rel err 0.0095 vs gate 0.02). NOTE: odd-length
# accumulation chains (13 matmuls) crash the device; keep NPAIR even.
# Every within-pair stride is positive; a8-plane pairs come first.
PAIRS14 = (
    [((HI_A, p), (LO_A, p + 1)) for p in range(KS * KS - 1)]
    + [((LO_A, 0), (HI_A, 8))]
    + [((HI_E, 2 * q), (HI_E, 2 * q + 1)) for q in range(4)]
    + [((HI_E, 8), (ZERO, 0))]
)
# 13-pair variant (comp pos 0 dropped, rel err 0.0135): runs as a 12-pair
# chain plus a separate 1-pair accumulation group, since 13-long chains
# crash the device.
PAIRS13 = (
    [((HI_A, p), (LO_A, p + 1)) for p in range(KS * KS - 1)]
    + [((LO_A, 0), (HI_A, 8))]
    + [((HI_E, 2 * q + 1), (HI_E, 2 * q + 2)) for q in range(4)]
)
PAIRS = PAIRS14
NPAIR = len(PAIRS)


def _moff(kind, pos, ohb, eoff, zoff):
    if kind == ZERO:
        return zoff
    kh, kw = pos // KS, pos % KS
    return (eoff if kind == HI_E else 0) + (kh + ohb * ROWS) * HP + kw


def _bfp_quantize_lastaxis(x):
    shape = x.shape
    xb = x.reshape(shape[:-1] + (shape[-1] // BLOCK, BLOCK)).astype(np.float32)
    maxabs = np.max(np.abs(xb), axis=-1, keepdims=True)
    exp = np.floor(np.log2(np.maximum(maxabs, np.float32(1e-38))))
    scale = np.exp2(exp - (M_BIT - 2)).astype(np.float32)
    qmax = np.float32(2.0 ** (M_BIT - 1) - 1)
    q = np.clip(np.round(xb / scale), -qmax - 1.0, qmax).astype(np.float32) * scale
    q = np.where(maxabs == 0.0, np.float32(0.0), q)
    return q.reshape(shape), np.repeat(
        scale.reshape(shape[:-1] + (shape[-1] // BLOCK,)), BLOCK, axis=-1
    )


_NC_CACHE = {}


def _build_program():
    if "nc" in _NC_CACHE:
        return _NC_CACHE["nc"]
    nc = bacc.Bacc("TRN2")
    fp8 = mybir.dt.float8e4
    f16 = mybir.dt.float16
    f32 = mybir.dt.float32

    WQ_SPLIT = bool(_NC_CACHE.get("wq_split", True))
    N_WARM = int(_NC_CACHE.get("n_warm", 16))
    USE_BAND = bool(_NC_CACHE.get("use_band", True))
    N_IMG_BUILD = int(_NC_CACHE.get("n_img", IMG_PER_CORE))
    USE_13 = bool(_NC_CACHE.get("use_13", True))
    pairs = PAIRS13 if USE_13 else PAIRS14
    npair = len(pairs)
    HEAD = (ROWS + 2) * HP
    BANDSZ = 2 * HEAD + ZPAD
    BANDOFF = IMG_PER_CORE * IMG_STRIDE

    xq = nc.dram_tensor("xq", [128, IMG_PER_CORE * IMG_STRIDE + 2 * BANDSZ],
                        fp8, kind="ExternalInput")
    if WQ_SPLIT:
        wq0 = nc.dram_tensor("wq0", [128, npair, 2, 128], fp8,
                             kind="ExternalInput")
        wq1 = nc.dram_tensor("wq1", [128, npair, 2, 128], fp8,
                             kind="ExternalInput")
    else:
        wq0 = nc.dram_tensor("wq0", [128, npair, 2, C_OUT], fp8,
                             kind="ExternalInput")
    outT = nc.dram_tensor("outT", [C_OUT, M], f16, kind="ExternalOutput")

    with TileContext(nc) as tc:
        with (
            tc.tile_pool(name="wpool", bufs=1) as wpool,
            tc.tile_pool(name="xpool", bufs=1) as xpool,
            tc.tile_pool(name="opool", bufs=3) as opool,
            tc.tile_pool(name="pspool", bufs=6, space="PSUM") as pspool,
        ):
            # PE warmup: dummy DoubleRow matmuls on a zeroed scratch tile keep
            # the tensor engine busy through its p-state ramp while the first
            # input/weight DMAs are in flight.
            if N_WARM:
                dummy = wpool.tile([128, 256], fp8, tag="dummy")
                nc.vector.memset(dummy[:, :], 0.0)
                dps = pspool.tile([128, MCHUNK], f32, tag="ps")
                dmov = AP(
                    dummy[:, :].tensor, 0,
                    [[dummy[:, :].ap[0][0], 128], [1, 2], [1, ROWS], [1, W]],
                )
                dw = AP(
                    dummy[:, :].tensor, 0,
                    [[dummy[:, :].ap[0][0], 128], [64, 2], [1, 128]],
                )
                for _ in range(N_WARM):
                    nc.tensor.matmul(
                        dps[:, :], dw, dmov, start=True, stop=True,
                        perf_mode=mybir.MatmulPerfMode.DoubleRow,
                    )

            # image-0 head bands: dedicated contiguous dram blocks
            # [a8 rows | ea8 rows | zeros] so each band is ONE small DMA
            # and the first chunks don't wait on the whole first image.
            # Startup DMAs alternate between the SP and ACT hwdge queues
            # (a dma_start holds its queue's SEQ for the full transfer).
            N_BANDS = int(_NC_CACHE.get("n_bands", 2)) if USE_BAND else 0

            def make_band(b, eng):
                xb = xpool.tile([128, BANDSZ], fp8, tag=f"xb{b}")
                off = BANDOFF + b * BANDSZ
                eng.dma_start(xb[:, :], xq[:, off : off + BANDSZ])
                return xb

            bands = []
            if N_BANDS:
                bands.append(make_band(0, nc.sync))
            if WQ_SPLIT:
                wt0 = wpool.tile([128, npair, 2, 128], fp8, tag="w0")
                nc.scalar.dma_start(wt0[:, :, :, :], wq0[:, :, :, :])
                wt1 = wpool.tile([128, npair, 2, 128], fp8, tag="w1")
                nc.sync.dma_start(wt1[:, :, :, :], wq1[:, :, :, :])

                def wslice(cb, j):
                    return (wt0 if cb == 0 else wt1)[:, j, :, :]
            else:
                wtc = wpool.tile([128, npair, 2, C_OUT], fp8, tag="w0")
                nc.sync.dma_start(wtc[:, :, :, :], wq0[:, :, :, :])

                def wslice(cb, j):
                    return wtc[:, j, :, cb * 128 : (cb + 1) * 128]
            for b in range(1, N_BANDS):
                bands.append(make_band(b, nc.scalar))
            xc = []
            for img in range(N_IMG_BUILD):
                xci = xpool.tile([128, IMG_STRIDE], fp8, tag=f"xc{img}")
                nc.sync.dma_start(
                    xci[:, :],
                    xq[:, img * IMG_STRIDE : (img + 1) * IMG_STRIDE],
                )
                xc.append(xci)

            for img in range(N_IMG_BUILD):
                for ohb in range(NOHB):
                    if img == 0 and ohb < N_BANDS:
                        base, eoff, zoff = bands[ohb][:, :], HEAD, 2 * HEAD
                        ohb_eff = 0
                    else:
                        base, eoff, zoff = xc[img][:, :], PLANE, ZOFF
                        ohb_eff = ohb
                    for cb in range(2):
                        ps = pspool.tile([128, MCHUNK], f32, tag="ps")
                        for j, ((k1, p1), (k2, p2)) in enumerate(pairs):
                            o1 = _moff(k1, p1, ohb_eff, eoff, zoff)
                            o2 = _moff(k2, p2, ohb_eff, eoff, zoff)
                            mov = AP(
                                base.tensor,
                                o1,
                                [[base.ap[0][0], 128], [o2 - o1, 2],
                                 [HP, ROWS], [1, W]],
                            )
                            nc.tensor.matmul(
                                ps[:, :],
                                wslice(cb, j),
                                mov,
                                start=(j == 0),
                                stop=(False if USE_13
                                      else j == npair - 1),
                                perf_mode=mybir.MatmulPerfMode.DoubleRow,
                            )
                        if USE_13:
                            # 14th instruction: 1-cycle all-zero DR matmul
                            # (13-instruction chains crash the device)
                            dz = dummy[:, :]
                            zw = AP(dz.tensor, 0,
                                    [[dz.ap[0][0], 128], [128, 2], [1, 128]])
                            zmov = AP(dz.tensor, 0,
                                      [[dz.ap[0][0], 128], [2, 2], [1, 2]])
                            nc.tensor.matmul(
                                ps[:, :2], zw, zmov,
                                start=False, stop=True,
                                perf_mode=mybir.MatmulPerfMode.DoubleRow,
                            )
                        ot = opool.tile([128, MCHUNK], f16, tag=f"o{cb}")
                        nc.vector.tensor_copy(ot[:, :], ps[:, :])
                        col = img * PIX + ohb * MCHUNK
                        (nc.sync if cb == 0 else nc.scalar).dma_start(
                            outT[cb * 128 : (cb + 1) * 128,
                                 col : col + MCHUNK],
                            ot[:, :],
                        )
    if not nc.is_finalized():
        nc.finalize()
    _NC_CACHE["nc"] = nc
    return nc


def _host_prep(inputs, weight, bias):
    x = np.asarray(inputs, dtype=np.float32)
    # padded activations + fp8 planes (quantized once per input pixel)
    xp = np.zeros((N_IMG, C_IN, HP, HP), dtype=np.float32)
    xp[:, :, 1:-1, 1:-1] = x
    a8 = xp.astype(FP8)
    ea8 = (xp - a8.astype(np.float32)).astype(FP8)

    HEAD = (ROWS + 2) * HP
    BANDSZ = 2 * HEAD + ZPAD
    xq_cores = []
    for c in range(N_CORES):
        arr = np.zeros((128, IMG_PER_CORE * IMG_STRIDE + 2 * BANDSZ),
                       dtype=FP8)
        av = arr[:, : IMG_PER_CORE * IMG_STRIDE].reshape(
            128, IMG_PER_CORE, IMG_STRIDE)
        sl = slice(c * IMG_PER_CORE, (c + 1) * IMG_PER_CORE)
        # [img, C, HP, HP] -> [C, img, PLANE]
        av[:, :, :PLANE] = a8[sl].reshape(
            IMG_PER_CORE, 128, PLANE).transpose(1, 0, 2)
        av[:, :, PLANE : 2 * PLANE] = ea8[sl].reshape(
            IMG_PER_CORE, 128, PLANE).transpose(1, 0, 2)
        # band blocks for image 0: [a8 rows 8b..8b+9 | ea8 rows | zeros]
        i0 = c * IMG_PER_CORE
        for b in range(2):
            off = IMG_PER_CORE * IMG_STRIDE + b * BANDSZ
            rows = slice(b * ROWS * HP, b * ROWS * HP + HEAD)
            arr[:, off : off + HEAD] = a8[i0].reshape(128, PLANE)[:, rows]
            arr[:, off + HEAD : off + 2 * HEAD] = (
                ea8[i0].reshape(128, PLANE)[:, rows])
        xq_cores.append(np.ascontiguousarray(arr))

    # weights: reference BFP quantization (exact), nibble split, 2^9 scale
    qw, ws = _bfp_quantize_lastaxis(
        np.asarray(weight, dtype=np.float32).reshape(C_OUT, K)
    )
    i = np.round(qw / ws)
    i_hi = np.round(i / 16.0)
    w_hi = (16.0 * i_hi * ws * WSCALE).astype(np.float32)
    w_lo = ((i - 16.0 * i_hi) * ws * WSCALE).astype(np.float32)
    # [C_OUT, K] -> [128 (c_in), 9 (pos), C_OUT]
    w_hi_t = w_hi.reshape(C_OUT, C_IN, KS * KS).transpose(1, 2, 0)
    w_lo_t = w_lo.reshape(C_OUT, C_IN, KS * KS).transpose(1, 2, 0)
    pairs = PAIRS13 if _NC_CACHE.get("use_13", True) else PAIRS14
    wq = np.zeros((128, len(pairs), 2, C_OUT), dtype=np.float32)
    for j, pair in enumerate(pairs):
        for slot, (kind, pos) in enumerate(pair):
            if kind == ZERO:
                continue
            src = w_lo_t if kind == LO_A else w_hi_t
            wq[:, j, slot, :] = src[:, pos, :]
    wq8 = wq.astype(FP8)
    assert np.array_equal(wq8.astype(np.float32), wq), "fp8 weight split inexact"
    wq8_0 = np.ascontiguousarray(wq8[:, :, :, :128])
    wq8_1 = np.ascontiguousarray(wq8[:, :, :, 128:])

    bias_f32 = np.asarray(bias, dtype=np.float32).reshape(C_OUT, 1)
    return xq_cores, wq8_0, wq8_1, bias_f32


def kernel(**inputs):
    xq_cores, wq8_0, wq8_1, bias_f32 = _host_prep(
        inputs["inputs"], inputs["weight"], inputs["bias"]
    )
    nc = _build_program()
    if _NC_CACHE.get("wq_split", True):
        in_maps = [
            {"xq": xq_cores[c], "wq0": wq8_0, "wq1": wq8_1}
            for c in range(N_CORES)
        ]
    else:
        wq8_c = np.ascontiguousarray(np.concatenate([wq8_0, wq8_1], axis=3))
        in_maps = [{"xq": xq_cores[c], "wq0": wq8_c} for c in range(N_CORES)]
    res = run_bass_kernel_spmd(nc, in_maps, core_ids=list(range(N_CORES)))
    outs = []
    for c in range(N_CORES):
        oT = res.results[c]["outT"].astype(np.float32) / WSCALE + bias_f32
        outs.append(oT.reshape(C_OUT, IMG_PER_CORE, PIX).transpose(1, 0, 2))
    out = np.concatenate(outs, axis=0).reshape(N_IMG, C_OUT, H, W)
    return np.ascontiguousarray(out.astype(np.float32))



# revision 16
# speedup vs baseline: 1.0786x; 1.0786x over previous
"""BFP-quantized 3x3 conv (stride 1, pad 1) as on-the-fly im2col matmul on
8 TRN2 cores, using fp8 DoubleRow matmuls (2 k-tiles per instruction at 0.5
cycles/row = 4x bf16 PE throughput).

Shapes (hardcoded): inputs [32,128,56,56] f32, weight [256,128,3,3] f32,
bias [256] f32 -> out [32,256,56,56] f32.

Strategy: data-parallel over batch (4 images per core). The reference
quantizes both operands to 8-bit-mantissa BFP; we approximate with 24 fp8
k-tiles per output (12 DoubleRow matmuls):

  out ~= sum_p a8 @ (w8[p] + ew8[p]) + sum_{p in COMP} ea8 @ w8[p]

where
  - w8 = e4m3(qw*512), ew8 = e4m3(qw*512 - w8): two-term fp8 expansion of
    the BFP-quantized weights (residual of the residual is ~2^-8 qw),
  - a8 = e4m3(x) quantized once per input pixel (so im2col can be done
    on the fly from shifted SBUF views -> no 9x HBM blowup),
  - ea8 = e4m3(x - a8) is an fp8 error-compensation plane applied on the
    6 COMP positions (3 dropped positions chosen at runtime to minimize a
    per-cout variance proxy of the realized weights).

Each DoubleRow matmul contracts 2 of the 24 k-tiles. PSUM accumulates in
f32; outputs stored f16 (scaled by 2^9), descaled + bias added on host.

Startup: the shared HWDGE stage costs ~630ns per DMA, so the first-chunk
data ([cb0 weights | band0 rows]) is fused into ONE per-core DMA; cb1
weights and the image-0 row band for the second chunk follow on alternating
queues, then the 4 full per-image blocks.
"""

import numpy as np
import ml_dtypes

import concourse.bacc as bacc
import concourse.mybir as mybir
from concourse.tile import TileContext
from concourse.bass_utils import run_bass_kernel_spmd
from bass_rust import AP

FP8 = ml_dtypes.float8_e4m3

N_CORES = 8
N_IMG, C_IN, H, W = 32, 128, 56, 56
C_OUT, KS = 256, 3
IMG_PER_CORE = N_IMG // N_CORES   # 4
PIX = H * W                       # 3136
M = IMG_PER_CORE * PIX            # 12544 output columns per core

HP = H + 2                        # 58 padded
PLANE = HP * HP                   # 3364 elements per partition per plane
IMG_STRIDE = 2 * PLANE            # 6728: [a8 plane | ea8 plane]

ROWS = 8                          # output rows per matmul chunk
MCHUNK = ROWS * W                 # 448 moving rows per DR matmul
NOHB = H // ROWS                  # 7 chunks per image

HEAD = (ROWS + 2) * HP            # 580 rows-per-band block
BANDSZ = 2 * HEAD                 # [a8 rows | ea8 rows]
NPAIR = 12
WBYTES = NPAIR * 2 * 128          # 3072 weight bytes per partition per cb

WSCALE = 512.0                    # global 2^9 weight scaling for fp8 range
M_BIT, BLOCK = 8, 64

# tile kinds: (plane, pos, wkind); plane 'A' = a8, 'E' = ea8;
# wkind 'W' = w8, 'V' = ew8 (weight residual)


def _make_pairs(drop):
    """24 tiles -> 12 DoubleRow pairs. drop: 3 positions whose ea8-comp
    tile is omitted. Within-pair offsets strictly increasing."""
    t1 = [("A", p, "W") for p in range(9)]
    t2 = [("A", p, "V") for p in range(9)]
    t3 = [("E", p, "W") for p in range(9) if p not in drop]
    # pair T1[p] with T2[(p+1) % 9]: distinct positions -> distinct offsets
    pairs = []
    for i in range(9):
        a, b = t1[i], t2[(i + 1) % 9]
        pairs.append((a, b) if i < 8 else (b, a))  # (T2[0], T1[8]) ordered
    assert len(t3) == 6
    for q in range(3):
        pairs.append((t3[2 * q], t3[2 * q + 1]))
    return pairs


def _moff(plane, pos, ohb, eoff):
    kh, kw = pos // KS, pos % KS
    return (eoff if plane == "E" else 0) + (kh + ohb * ROWS) * HP + kw


def _bfp_quantize_lastaxis(x):
    shape = x.shape
    xb = x.reshape(shape[:-1] + (shape[-1] // BLOCK, BLOCK)).astype(np.float32)
    maxabs = np.max(np.abs(xb), axis=-1, keepdims=True)
    exp = np.floor(np.log2(np.maximum(maxabs, np.float32(1e-38))))
    scale = np.exp2(exp - (M_BIT - 2)).astype(np.float32)
    qmax = np.float32(2.0 ** (M_BIT - 1) - 1)
    q = np.clip(np.round(xb / scale), -qmax - 1.0, qmax).astype(np.float32) * scale
    q = np.where(maxabs == 0.0, np.float32(0.0), q)
    return q.reshape(shape)


_NC_CACHE = {}


def _build_program(drop=None):
    if drop is None:
        drop = _NC_CACHE.get("last_drop", (0, 4, 8))
    drop = tuple(sorted(drop))
    _NC_CACHE["last_drop"] = drop
    key = ("nc", drop)
    if key in _NC_CACHE:
        return _NC_CACHE[key]
    nc = bacc.Bacc("TRN2")
    fp8 = mybir.dt.float8e4
    f16 = mybir.dt.float16
    f32 = mybir.dt.float32

    N_WARM = int(_NC_CACHE.get("n_warm", 15))
    PS_BUFS = int(_NC_CACHE.get("ps_bufs", 8))
    O_BUFS = int(_NC_CACHE.get("o_bufs", 6))
    pairs = _make_pairs(set(drop))
    npair = len(pairs)
    BANDOFF = IMG_PER_CORE * IMG_STRIDE

    FUSE_W0 = bool(_NC_CACHE.get("fuse_w0", True))
    if FUSE_W0:
        # per-core [cb0 weights | band0 a8 rows | band0 ea8 rows]
        wq0 = nc.dram_tensor("wq0", [128, WBYTES + BANDSZ], fp8,
                             kind="ExternalInput")
        n_tail_bands = 1
    else:
        wq0 = nc.dram_tensor("wq0", [128, npair, 2, 128], fp8,
                             kind="ExternalInput")
        n_tail_bands = 2
    wq1 = nc.dram_tensor("wq1", [128, npair, 2, 128], fp8,
                         kind="ExternalInput")
    # per-core activations: 4 image blocks + band tail(s)
    xq = nc.dram_tensor("xq", [128, BANDOFF + n_tail_bands * BANDSZ], fp8,
                        kind="ExternalInput")
    outT = nc.dram_tensor("outT", [C_OUT, M], f16, kind="ExternalOutput")

    with TileContext(nc) as tc:
        with (
            tc.tile_pool(name="wpool", bufs=1) as wpool,
            tc.tile_pool(name="xpool", bufs=1) as xpool,
            tc.tile_pool(name="opool", bufs=O_BUFS) as opool,
            tc.tile_pool(name="pspool", bufs=PS_BUFS, space="PSUM") as pspool,
        ):
            # PE warmup: dummy DoubleRow matmuls on a zeroed scratch tile keep
            # the tensor engine busy through its p-state ramp while the first
            # input/weight DMAs are in flight.
            dummy = wpool.tile([128, 256], fp8, tag="dummy")
            nc.vector.memset(dummy[:, :], 0.0)
            dps = pspool.tile([128, MCHUNK], f32, tag="ps")
            dmov = AP(
                dummy[:, :].tensor, 0,
                [[dummy[:, :].ap[0][0], 128], [1, 2], [1, ROWS], [1, W]],
            )
            dw = AP(
                dummy[:, :].tensor, 0,
                [[dummy[:, :].ap[0][0], 128], [64, 2], [1, 128]],
            )
            for _ in range(N_WARM):
                nc.tensor.matmul(
                    dps[:, :], dw, dmov, start=True, stop=True,
                    perf_mode=mybir.MatmulPerfMode.DoubleRow,
                )

            # startup DMAs in need order; shared-HWDGE cost ~630ns each, so
            # the first chunk's data is ONE fused transfer when fuse_w0.
            if FUSE_W0:
                wb0 = wpool.tile([128, WBYTES + BANDSZ], fp8, tag="w0")
                nc.sync.dma_start(wb0[:, :], wq0[:, :])
            else:
                wt0 = wpool.tile([128, npair, 2, 128], fp8, tag="w0")
                nc.sync.dma_start(wt0[:, :, :, :], wq0[:, :, :, :])
                band0 = xpool.tile([128, BANDSZ], fp8, tag="xb0")
                boff0 = BANDOFF + BANDSZ
                nc.scalar.dma_start(band0[:, :], xq[:, boff0:boff0 + BANDSZ])
            wt1 = wpool.tile([128, npair, 2, 128], fp8, tag="w1")
            (nc.scalar if FUSE_W0 else nc.sync).dma_start(
                wt1[:, :, :, :], wq1[:, :, :, :])
            band1 = xpool.tile([128, BANDSZ], fp8, tag="xb1")
            nc.sync.dma_start(band1[:, :], xq[:, BANDOFF:BANDOFF + BANDSZ])
            xc = []
            for img in range(IMG_PER_CORE):
                xci = xpool.tile([128, IMG_STRIDE], fp8, tag=f"xc{img}")
                (nc.scalar if img % 2 == 0 else nc.sync).dma_start(
                    xci[:, :],
                    xq[:, img * IMG_STRIDE:(img + 1) * IMG_STRIDE],
                )
                xc.append(xci)

            def wslice(cb, j):
                if cb == 1:
                    return wt1[:, j, :, :]
                if not FUSE_W0:
                    return wt0[:, j, :, :]
                v = wb0[:, :]
                return AP(v.tensor, j * 256,
                          [[v.ap[0][0], 128], [128, 2], [1, 128]])

            TAIL_SPLIT = int(_NC_CACHE.get("tail_split", 0))
            CHAIN_PAD = int(_NC_CACHE.get("chain_pad", 0))

            def do_chunk(img, ohb, cb, row0, nrows):
                """One matmul chain + copy + store for `nrows` output rows
                starting at `row0` within the image."""
                if img == 0 and ohb < 2:
                    if ohb == 0:
                        if FUSE_W0:
                            base, boff, eoff = wb0[:, :], WBYTES, HEAD
                        else:
                            base, boff, eoff = band0[:, :], 0, HEAD
                    else:
                        base, boff, eoff = band1[:, :], 0, HEAD
                    r0 = row0 - ohb * ROWS
                else:
                    base, boff, eoff = xc[img][:, :], 0, PLANE
                    r0 = row0
                ncols = nrows * W
                ps = pspool.tile([128, MCHUNK], f32, tag="ps")
                for j, (t1, t2) in enumerate(pairs):
                    o1 = boff + r0 * HP + _moff(t1[0], t1[1], 0, eoff)
                    o2 = boff + r0 * HP + _moff(t2[0], t2[1], 0, eoff)
                    mov = AP(
                        base.tensor,
                        o1,
                        [[base.ap[0][0], 128], [o2 - o1, 2],
                         [HP, nrows], [1, W]],
                    )
                    nc.tensor.matmul(
                        ps[:, :ncols],
                        wslice(cb, j),
                        mov,
                        start=(j == 0),
                        stop=(j == npair - 1 and not CHAIN_PAD),
                        perf_mode=mybir.MatmulPerfMode.DoubleRow,
                    )
                for q in range(CHAIN_PAD):
                    # 1-cycle all-zero DR matmuls padding the chain length
                    dz = dummy[:, :]
                    zw = AP(dz.tensor, 0,
                            [[dz.ap[0][0], 128], [128, 2], [1, 128]])
                    zmov = AP(dz.tensor, 0,
                             [[dz.ap[0][0], 128], [2, 2], [1, 2]])
                    nc.tensor.matmul(
                        ps[:, :2], zw, zmov,
                        start=False, stop=(q == CHAIN_PAD - 1),
                        perf_mode=mybir.MatmulPerfMode.DoubleRow,
                    )
                ot = opool.tile([128, MCHUNK], f16, tag=f"o{cb}")
                nc.vector.tensor_copy(ot[:, :ncols], ps[:, :ncols])
                col = img * PIX + row0 * W
                (nc.sync if cb == 0 else nc.scalar).dma_start(
                    outT[cb * 128:(cb + 1) * 128, col:col + ncols],
                    ot[:, :ncols],
                )

            for img in range(IMG_PER_CORE):
                for ohb in range(NOHB):
                    last = img == IMG_PER_CORE - 1 and ohb == NOHB - 1
                    if last and TAIL_SPLIT:
                        # final row-band split so the tail-critical
                        # copy+DMA after the very last matmul is small
                        for cb in range(2):
                            do_chunk(img, ohb, cb,
                                     ohb * ROWS, ROWS - TAIL_SPLIT)
                        for cb in range(2):
                            do_chunk(img, ohb, cb,
                                     ohb * ROWS + ROWS - TAIL_SPLIT,
                                     TAIL_SPLIT)
                    else:
                        for cb in range(2):
                            do_chunk(img, ohb, cb, ohb * ROWS, ROWS)
    if not nc.is_finalized():
        nc.finalize()
    _NC_CACHE[key] = nc
    return nc


def _host_prep(inputs, weight, bias):
    x = np.asarray(inputs, dtype=np.float32)
    # padded activations + fp8 planes (quantized once per input pixel)
    xp = np.zeros((N_IMG, C_IN, HP, HP), dtype=np.float32)
    xp[:, :, 1:-1, 1:-1] = x
    a8 = xp.astype(FP8)
    ea8 = (xp - a8.astype(np.float32)).astype(FP8)

    # weights: reference BFP quantization, then two-term e4m3 expansion
    qw = _bfp_quantize_lastaxis(
        np.asarray(weight, dtype=np.float32).reshape(C_OUT, C_IN * KS * KS)
    )
    # [C_OUT, K] -> [128 (c_in), 9 (pos), C_OUT]
    qw_t = qw.reshape(C_OUT, C_IN, KS * KS).transpose(1, 2, 0) * WSCALE
    w8 = qw_t.astype(FP8).astype(np.float32)
    ew8 = (qw_t - w8).astype(FP8).astype(np.float32)

    # drop the 3 ea8-comp positions with the smallest worst-cout error
    # variance proxy: max_cout sum_c w8^2 (x var(ea8) const across p).
    score = np.max(np.sum(w8 * w8, axis=0), axis=1)  # [9]
    drop = tuple(sorted(np.argsort(score)[:3].tolist()))

    pairs = _make_pairs(set(drop))
    wq = np.zeros((128, len(pairs), 2, C_OUT), dtype=np.float32)
    for j, pair in enumerate(pairs):
        for slot, (plane, pos, wkind) in enumerate(pair):
            wq[:, j, slot, :] = w8[:, pos, :] if wkind == "W" else ew8[:, pos, :]
    wq8 = wq.astype(FP8)
    wq8_1 = np.ascontiguousarray(wq8[:, :, :, 128:])

    fuse = bool(_NC_CACHE.get("fuse_w0", True))
    n_tail_bands = 1 if fuse else 2
    xq_cores, wq0_cores = [], []
    for c in range(N_CORES):
        arr = np.zeros(
            (128, IMG_PER_CORE * IMG_STRIDE + n_tail_bands * BANDSZ),
            dtype=FP8)
        av = arr[:, : IMG_PER_CORE * IMG_STRIDE].reshape(
            128, IMG_PER_CORE, IMG_STRIDE)
        sl = slice(c * IMG_PER_CORE, (c + 1) * IMG_PER_CORE)
        # [img, C, HP, HP] -> [C, img, PLANE]
        av[:, :, :PLANE] = a8[sl].reshape(
            IMG_PER_CORE, 128, PLANE).transpose(1, 0, 2)
        av[:, :, PLANE:] = ea8[sl].reshape(
            IMG_PER_CORE, 128, PLANE).transpose(1, 0, 2)
        i0 = c * IMG_PER_CORE
        a0 = a8[i0].reshape(128, PLANE)
        e0 = ea8[i0].reshape(128, PLANE)
        # band tails: image-0 rows [8..17] (band1) and, unfused, [0..9]
        off = IMG_PER_CORE * IMG_STRIDE
        rows = slice(ROWS * HP, ROWS * HP + HEAD)
        arr[:, off:off + HEAD] = a0[:, rows]
        arr[:, off + HEAD:off + BANDSZ] = e0[:, rows]
        if not fuse:
            off += BANDSZ
            arr[:, off:off + HEAD] = a0[:, :HEAD]
            arr[:, off + HEAD:off + BANDSZ] = e0[:, :HEAD]
        xq_cores.append(np.ascontiguousarray(arr))
        if fuse:
            # per-core fused [cb0 weights | band0 rows 0..9 [a8 | ea8]]
            warr = np.zeros((128, WBYTES + BANDSZ), dtype=FP8)
            warr[:, :WBYTES] = wq8[:, :, :, :128].reshape(128, WBYTES)
            warr[:, WBYTES:WBYTES + HEAD] = a0[:, :HEAD]
            warr[:, WBYTES + HEAD:] = e0[:, :HEAD]
            wq0_cores.append(np.ascontiguousarray(warr))
    if not fuse:
        wq0_shared = np.ascontiguousarray(wq8[:, :, :, :128])
        wq0_cores = [wq0_shared] * N_CORES

    bias_f32 = np.asarray(bias, dtype=np.float32).reshape(C_OUT, 1)
    return xq_cores, wq0_cores, wq8_1, bias_f32, drop


def kernel(**inputs):
    xq_cores, wq0_cores, wq8_1, bias_f32, drop = _host_prep(
        inputs["inputs"], inputs["weight"], inputs["bias"]
    )
    nc = _build_program(drop)
    in_maps = [
        {"xq": xq_cores[c], "wq0": wq0_cores[c], "wq1": wq8_1}
        for c in range(N_CORES)
    ]
    res = run_bass_kernel_spmd(nc, in_maps, core_ids=list(range(N_CORES)))
    outs = []
    for c in range(N_CORES):
        oT = res.results[c]["outT"].astype(np.float32) / WSCALE + bias_f32
        outs.append(oT.reshape(C_OUT, IMG_PER_CORE, PIX).transpose(1, 0, 2))
    out = np.concatenate(outs, axis=0).reshape(N_IMG, C_OUT, H, W)
    return np.ascontiguousarray(out.astype(np.float32))
